# revision 8
# baseline (speedup 1.0000x reference)
"""Trainium2 Bass kernel for single-head causal attention.

Problem: x[4, 4096, 100], Wq/Wk/Wv[100, 64] ->
         softmax(tril(x@Wq @ (x@Wk)^T / 8)) @ (x@Wv)   -> [4, 4096, 64]

Sharding (8 cores, SPMD single program):
  core = 2*b + c: batch b in 0..3, key-parity c in 0..1.
  Each core handles ALL 4096 queries of its batch but only the keys/values at
  global rows {c, c+2, c+4, ...} (2048 of them). This keeps the causal
  structure IDENTICAL across cores (local key tile kk is attended by global
  query columns q >= 256*kk, for both parities), so one program serves all 8
  cores; the one-column parity offset lives in a tiny [128, 256] data mask.
  Softmax is computed without max-subtraction (scores are bounded ~|s|<=9
  after the 1/8 scale, exp can't overflow), so the two half-key partials
  combine on the host as (num_A + num_B) / (den_A + den_B).

Per-core program (flash-attention style, scores kept transposed):
  qT = Wq^T x^T  [64, 4096],  kT = Wk^T xkv^T [64, 2048]  (fp32r matmuls)
  V1[kk] = [x_kv@Wv | 1] per 128-key tile (bf16, ones col -> denominator)
  for each 512-query block qm (8 blocks):
    for each key tile kk in 0..2*qm+1 (grouped by 3 for big exp calls):
      S^T[kk] = kT[kk]^T-block @ qT-block   (fp32r, PSUM [128, 512])
      E = exp(S^T / 8)  (ACT engine, PSUM->SBUF bf16, one call per group)
      boundary tiles: E *= mask (DVE), lower-256 of the last tile zeroed
      out' += V1[kk]^T @ E  (bf16 matmul, accumulates [65, 512] in PSUM)
    flush out' -> SBUF -> DRAM out[65, 4096]  (rows 0..63 = sum exp*v, row 64 = sum exp)
"""

import os
from contextlib import ExitStack

import numpy as np

B, T, E, H = 4, 4096, 100, 64
TK = T // 2  # keys per core
NKT = TK // 128  # 16 local key tiles
NQB = T // 512  # 8 query blocks
N_CORES = 8

_CACHE = {}


def _mask_np(c):
    """mask[i, j] = 1 if global key (2i+c) <= query col offset j else 0."""
    import ml_dtypes

    i = np.arange(128)[:, None]
    j = np.arange(256)[None, :]
    return (j >= 2 * i + c).astype(ml_dtypes.bfloat16)


def _build():
    if "nc" in _CACHE:
        return _CACHE["nc"]

    import concourse.bacc as bacc
    import concourse.tile as tile
    from concourse import mybir
    from concourse.bass import ts, ds

    f32 = mybir.dt.float32
    f32r = mybir.dt.float32r
    bf16 = mybir.dt.bfloat16
    Exp = mybir.ActivationFunctionType.Exp
    Mult = mybir.AluOpType.mult

    nc = bacc.Bacc("TRN2", target_bir_lowering=False, debug=False,
                   num_devices=N_CORES)

    xq_d = nc.dram_tensor("xq", [128, T], f32r, kind="ExternalInput").ap()
    xkv_d = nc.dram_tensor("xkv", [128, TK], f32r, kind="ExternalInput").ap()
    wq_d = nc.dram_tensor("wq", [128, H], f32r, kind="ExternalInput").ap()
    wk_d = nc.dram_tensor("wk", [128, H], f32r, kind="ExternalInput").ap()
    wv_d = nc.dram_tensor("wv", [128, H], f32r, kind="ExternalInput").ap()
    mask_d = nc.dram_tensor("mask", [128, 256], bf16, kind="ExternalInput").ap()
    out_d = nc.dram_tensor("out", [H + 1, T], f32, kind="ExternalOutput").ap()

    with tile.TileContext(nc) as tc, ExitStack() as ctx:
        sb = ctx.enter_context(tc.tile_pool(name="sb", bufs=1))
        ep = ctx.enter_context(tc.tile_pool(name="ep", bufs=3))
        ob_p = ctx.enter_context(tc.tile_pool(name="ob", bufs=2))
        # PSUM budget (8 banks): tag "s" 3x[128,2,512] = 6 (shared by pair
        # strips AND projection outputs), tag "o" 2x[128,512] = 2.
        psA = ctx.enter_context(tc.tile_pool(name="psA", bufs=3, space="PSUM"))
        ps_o = ctx.enter_context(tc.tile_pool(name="ps_o", bufs=2, space="PSUM"))

        # E=100 inputs arrive zero-padded to 128 partitions from the host:
        # K=128 matmuls keep the PE's HAM activity monitor fed (K<128 matmuls
        # never un-throttle the PE clock from 1.2 to 2.4 GHz).
        xq_t = sb.tile([128, T], f32r)
        xkv_t = sb.tile([128, TK], f32r)
        wq_t = sb.tile([128, H], f32r)
        wk_t = sb.tile([128, H], f32r)
        wv_t = sb.tile([128, H], f32r)
        mask_t = sb.tile([128, 256], bf16)
        # qT/kT live duplicated in both partition halves (rows 0:64 == 64:128)
        # so S^T matmuls for key tiles (kk, kk+1) run CONCURRENTLY in the two
        # halves of the PE array (tile_position row groups).
        qT_t = sb.tile([128, T], f32r)
        kT_t = sb.tile([128, TK], f32r)
        v1_t = sb.tile([128, NKT, H + 1], bf16)
        warm_t = sb.tile([128, 8], f32)

        # DMA order = first-use order: weights/mask, then chunk 0 of each x.
        nc.sync.dma_start(out=wq_t, in_=wq_d)
        nc.sync.dma_start(out=wk_t, in_=wk_d)
        nc.sync.dma_start(out=wv_t, in_=wv_d)
        nc.sync.dma_start(out=mask_t, in_=mask_d)
        nc.sync.dma_start(out=xq_t[:, 0:512], in_=xq_d[:, 0:512])
        nc.sync.dma_start(out=xkv_t[:, 0:512], in_=xkv_d[:, 0:512])
        nc.sync.dma_start(out=xq_t[:, 512:2048], in_=xq_d[:, 512:2048])
        nc.sync.dma_start(out=xkv_t[:, 512:2048], in_=xkv_d[:, 512:2048])
        nc.sync.dma_start(out=xq_t[:, 2048:4096], in_=xq_d[:, 2048:4096])

        # First ACT instruction early: overlaps the ~2.7us exp-table load
        # with input DMA.
        nc.vector.memset(warm_t, 0.0)
        nc.scalar.activation(out=warm_t, in_=warm_t, func=Exp)
        nc.vector.memset(v1_t[:, :, H], 1.0)

        # ---- lazy projections, emitted just-in-time inside the main loop so
        # the PE has filler work while the ACT engine is the steady-state
        # bottleneck, and the first attention pair starts ~10us earlier.
        qT_done = [False] * (T // 512)
        kT_done = [False] * (TK // 512)
        v_done = [False] * NKT

        def need_qT(j):
            if qT_done[j]:
                return
            qT_done[j] = True
            ps = psA.tile([128, 512], f32, tag="s")
            nc.tensor.matmul(ps[:H], wq_t, xq_t[:, ts(j, 512)],
                             start=True, stop=True)
            nc.vector.tensor_copy(qT_t[:H, ts(j, 512)], ps[:H])
            nc.gpsimd.dma_start(out=qT_t[H:128, ts(j, 512)],
                                in_=qT_t[:H, ts(j, 512)])

        def need_kT(j):
            if kT_done[j]:
                return
            kT_done[j] = True
            ps = psA.tile([128, 512], f32, tag="s")
            nc.tensor.matmul(ps[:H], wk_t, xkv_t[:, ts(j, 512)],
                             start=True, stop=True)
            nc.vector.tensor_copy(kT_t[:H, ts(j, 512)], ps[:H])
            nc.gpsimd.dma_start(out=kT_t[H:128, ts(j, 512)],
                                in_=kT_t[:H, ts(j, 512)])

        def need_v(kk):
            if v_done[kk]:
                return
            v_done[kk] = True
            ps = psA.tile([128, 512], f32, tag="s")
            nc.tensor.matmul(ps[:, :H], xkv_t[:, ts(kk, 128)], wv_t,
                             start=True, stop=True)
            nc.vector.tensor_copy(v1_t[:, kk, :H], ps[:, :H])

        # ---- main attention loop ----
        # Software pipeline: each pair's AV matmuls are emitted one pair LATE
        # in program order, so the (in-order) PE runs the next pair's S^T
        # matmuls while the ACT engine computes this pair's exp.  S^T matmuls
        # are issued in PAIRS on row groups 0:64 / 64:128 of the PE array:
        # two K=64 matmuls run concurrently (~2x) and together keep the HAM
        # activity monitor fed so the PE clock stays at 2.4 GHz.
        def emit_av(p):
            e_t, pr, o_t, nkk, qm, packed = p
            kk = pr[0]
            if packed:
                # boundary pair, B's valid 256 cols packed at e_t[:, 1, :256]
                nc.tensor.matmul(o_t[:H + 1, 256:512], v1_t[:, kk + 1],
                                 e_t[:, 1, :256], start=False, stop=False)
                nc.tensor.matmul(o_t[:H + 1], v1_t[:, kk], e_t[:, 0],
                                 start=False, stop=True)
            else:
                nc.tensor.matmul(o_t[:H + 1], v1_t[:, kk], e_t[:, 0],
                                 start=(kk == 0), stop=False)
                nc.tensor.matmul(o_t[:H + 1], v1_t[:, kk + 1], e_t[:, 1],
                                 start=False, stop=(kk + 1 == nkk - 1))
            if pr[-1] == nkk - 1:  # last pair of qm: flush out'
                ob = ob_p.tile([H + 1, 512], f32, tag="ob")
                nc.vector.tensor_copy(ob, o_t[:H + 1])
                nc.sync.dma_start(out=out_d[:, ds(512 * qm, 512)], in_=ob)

        pend = None
        for qm in range(NQB):
            nkk = 2 * qm + 2
            need_qT(qm)
            for j in range((nkk - 1) // 4 + 1):
                need_kT(j)
            for kk in range(nkk):
                need_v(kk)
            o_t = ps_o.tile([128, 512], f32, tag="o")
            qs_lo = qT_t[:H, ds(512 * qm, 512)]
            qs_hi = qT_t[H:128, ds(512 * qm, 512)]
            for kk in range(0, nkk, 2):
                boundary = kk == 2 * qm
                packed = boundary and qm >= 1
                s_t = psA.tile([128, 2, 512], f32, tag="s")
                e_t = ep.tile([128, 2, 512], bf16, tag="e")
                nc.tensor.matmul(s_t[:, 0], kT_t[:H, ts(kk, 128)], qs_lo,
                                 start=True, stop=True)
                if packed:
                    # B attends only q cols [256:512); compute just those,
                    # packed to the left, and exp 768 cols instead of 1024.
                    nc.tensor.matmul(s_t[:, 1, :256],
                                     kT_t[H:128, ts(kk + 1, 128)],
                                     qs_hi[:, 256:512], start=True, stop=True)
                    nc.scalar.activation(
                        out=e_t.rearrange("p a b -> p (a b)")[:, :768],
                        in_=s_t.rearrange("p a b -> p (a b)")[:, :768],
                        func=Exp, scale=float(H) ** -0.5)
                    nc.vector.tensor_tensor(e_t[:, 0, 0:256],
                                            e_t[:, 0, 0:256], mask_t, Mult)
                    nc.vector.tensor_tensor(e_t[:, 1, 0:256],
                                            e_t[:, 1, 0:256], mask_t, Mult)
                else:
                    nc.tensor.matmul(s_t[:, 1], kT_t[H:128, ts(kk + 1, 128)],
                                     qs_hi, start=True, stop=True)
                    nc.scalar.activation(out=e_t, in_=s_t,
                                         func=Exp, scale=float(H) ** -0.5)
                    if boundary:  # qm == 0: pair (0,1) is first AND boundary
                        nc.vector.tensor_tensor(e_t[:, 0, 0:256],
                                                e_t[:, 0, 0:256], mask_t, Mult)
                        nc.vector.tensor_tensor(e_t[:, 1, 256:512],
                                                e_t[:, 1, 256:512], mask_t,
                                                Mult)
                        nc.vector.memset(e_t[:, 1, 0:256], 0.0)
                if pend is not None:
                    emit_av(pend)
                pend = (e_t, (kk, kk + 1), o_t, nkk, qm, packed)
        emit_av(pend)

    nc.compile()
    _CACHE["nc"] = nc
    return nc


def _tf32(a):
    """Round fp32 to tf32 (RNE to 10 mantissa bits) — fp32r's storage format."""
    u = np.ascontiguousarray(a, dtype=np.float32).view(np.uint32)
    u = (u + np.uint32(0xFFF) + ((u >> np.uint32(13)) & np.uint32(1))) \
        & np.uint32(0xFFFFE000)
    return u.view(np.float32)


def _pad128(a):
    """Zero-pad the leading (embedding) dim from 100 to 128 rows."""
    out = np.zeros((128,) + a.shape[1:], dtype=a.dtype)
    out[:a.shape[0]] = a
    return out


def _make_in_maps(x, Wq, Wk, Wv):
    x = np.asarray(x, dtype=np.float32)
    Wq = np.asarray(Wq, dtype=np.float32)
    Wk = np.asarray(Wk, dtype=np.float32)
    Wv = np.asarray(Wv, dtype=np.float32)
    masks = [_mask_np(0), _mask_np(1)]
    in_maps = []
    for core in range(N_CORES):
        b, c = divmod(core, 2)
        in_maps.append({
            "xq": _pad128(_tf32(x[b].T)),
            "xkv": _pad128(_tf32(x[b, c::2, :].T)),
            "wq": _pad128(_tf32(Wq)), "wk": _pad128(_tf32(Wk)),
            "wv": _pad128(_tf32(Wv)),
            "mask": masks[c],
        })
    return in_maps


def _combine(results):
    out = np.empty((B, T, H), dtype=np.float32)
    for b in range(B):
        a = results[2 * b]["out"]
        bb = results[2 * b + 1]["out"]
        num = a[:H] + bb[:H]
        den = a[H] + bb[H]
        out[b] = (num / den).T
    return out


def run(x, Wq, Wk, Wv, trace=False):
    """Returns (output [4,4096,64] f32, exec_time_ns or None)."""
    from concourse.bass_utils import run_bass_kernel_spmd

    nc = _build()
    in_maps = _make_in_maps(x, Wq, Wk, Wv)
    res = run_bass_kernel_spmd(nc, in_maps, core_ids=list(range(N_CORES)),
                               trace=trace)
    return _combine(res.results), res


def kernel(x, Wq, Wk, Wv):
    out, _ = run(x, Wq, Wk, Wv, trace=False)
    return out
